# revision 22
# baseline (speedup 1.0000x reference)
"""Trainium2 Bass kernel for nn_MessageLayer (GNN message passing).

Reference computation (per edge, E=1.6M, H=16, DE=32):
    A = (e @ W1 + b1).reshape(E, 16, 16)
    out[e,i] = sum_j A[e,i,j] * h[e,j]  +  (e @ W2 + b2)[e,i]

Pure data-parallel over E across 8 cores. Per core, edges are processed in
blocks of 4096 = 8 chunks x F=512 (one PSUM bank of fp32 per matmul output).

Partition layout ("j-major"): SBUF/PSUM partition p <-> (j, il) with
j = p//8, il = p%8.  Per chunk (512 edges):

  PE:  pa[:,0:512]   = wa^T @ x[0:65]   A^T half a (i=il),    PSUM bank pair
       pa[:,512:1024]= wb^T @ x[0:65]   A^T half b (i=il+8)
  t = pa * sh over [128,1024]; sh[p] = h[p//8] is read twice per chunk via a
       stride-0 middle AP dim. 1 of 3 chunks: direct DVE tensor_tensor from
       PSUM (1x); else ACT first evacuates pa to fp16 SBUF so the DVE
       multiply runs in 2x mode -- balances DVE vs ACT busy time.
  PE into pm [128,512] (24 matmuls per block accumulate, chunk c writes
       partition group 16c..16c+15 via shifted indicator weights):
       wcomb_c^T @ x[0:65]  (e@W2 + b1-term via h rows + b2 via ones row)
     + ga_c^T @ t[:,0:512]  (sum_j over half a)
     + gb_c^T @ t[:,512:]   (sum_j over half b)
  ACT: mo = copy(pm) fp16; one [128,512] copy + one DMA per 4096 edges.

All contraction dims are zero-padded to 65 (the pad rows of x are constant,
DMA'd once from a DRAM constant) and output dims to 128 so every matmul has
tile_size (128,128) -- no PE tiling-mode switches. Input DMAs are issued
from the SP queue, the sh broadcast (one DMA, stride-0 replication) from the
Pool queue, output copy + DMA from the ACT queue.

fp16 operands on the PE (fp32 matmul is 4x slower); PSUM accumulation fp32;
fp16 output (host upcasts). Measured rel-l2 error vs fp32 reference ~4e-4.
"""

import numpy as np

import concourse.mybir as mybir
import concourse.tile as tile
from concourse import bacc
from concourse.ap import AP
from concourse.bass_utils import run_bass_kernel_spmd

H = 16
DE = 32
NCORES = 8
F = 512          # edges per chunk (one PSUM bank of fp32)
BC = 8           # chunks per block
BLK = BC * F     # 4096 edges per block
KP = 65          # padded contraction dim: 32 e + 16 h + 1 ones + 16 zeros

f16 = mybir.dt.float16
f32 = mybir.dt.float32


def _inject_dim(ap, dim):
    """Return a copy of `ap` with `dim` ([stride, size]) inserted after the
    partition dim."""
    dims = [list(d) for d in ap.ap]
    return AP(ap.tensor, ap.offset, [dims[0], dim] + dims[1:])


def build_program(nblk: int):
    """SPMD Bass program for one core processing nblk*BLK edges."""
    Ec = nblk * BLK
    nc = bacc.Bacc("TRN2", target_bir_lowering=False, debug=False)

    eT_d = nc.dram_tensor("eT", [DE, Ec], f16, kind="ExternalInput")
    hT_d = nc.dram_tensor("hT", [H, Ec], f16, kind="ExternalInput")
    wa_d = nc.dram_tensor("wa", [KP, 128], f16, kind="ExternalInput")
    wb_d = nc.dram_tensor("wb", [KP, 128], f16, kind="ExternalInput")
    wcomb_d = nc.dram_tensor("wcomb", [KP, BC * 128], f16, kind="ExternalInput")
    ga_d = nc.dram_tensor("ga", [128, BC * 128], f16, kind="ExternalInput")
    gb_d = nc.dram_tensor("gb", [128, BC * 128], f16, kind="ExternalInput")
    pad_d = nc.dram_tensor("pad", [KP - DE - H, BLK], f16, kind="ExternalInput")
    mT_d = nc.dram_tensor("mT", [128, Ec // BC], f16, kind="ExternalOutput")

    mul = mybir.AluOpType.mult

    XBUFS = 3

    with tile.TileContext(nc) as tc:
        with (
            tc.tile_pool(name="const", bufs=1) as cpool,
            tc.tile_pool(name="sh", bufs=3) as shpool,
            tc.tile_pool(name="t", bufs=3) as tpool,
            tc.tile_pool(name="mo", bufs=2) as mopool,
            tc.tile_pool(name="pa", bufs=3, space="PSUM") as papool,
            tc.tile_pool(name="pm", bufs=2, space="PSUM") as pmpool,
        ):
            wa_s = cpool.tile([KP, 128], f16, tag="wa")
            wb_s = cpool.tile([KP, 128], f16, tag="wb")
            wcomb_s = cpool.tile([KP, BC * 128], f16, tag="wcomb")
            ga_s = cpool.tile([128, BC * 128], f16, tag="ga")
            gb_s = cpool.tile([128, BC * 128], f16, tag="gb")
            nc.scalar.dma_start(wa_s[:], wa_d[:])
            nc.scalar.dma_start(wb_s[:], wb_d[:])
            nc.scalar.dma_start(wcomb_s[:], wcomb_d[:])
            nc.scalar.dma_start(ga_s[:], ga_d[:])
            nc.scalar.dma_start(gb_s[:], gb_d[:])

            # x buffers: one persistent tile, XBUFS manually-rotated block
            # segments. Rows 48 (ones) and 49-64 (zeros) are constant pad --
            # initialized once, never rewritten, so every matmul can take
            # rhs = x[0:65] and run in the uniform (128,128) tile mode.
            x_all = cpool.tile([KP, XBUFS * BLK], f16, tag="x_all")
            for s in range(XBUFS):
                nc.scalar.dma_start(
                    x_all[DE + H : KP, s * BLK : (s + 1) * BLK], pad_d[:]
                )

            for b in range(nblk):
                bsl = slice(b * BLK, (b + 1) * BLK)
                seg = (b % XBUFS) * BLK
                nc.sync.dma_start(x_all[0:DE, seg : seg + BLK], eT_d[:, bsl])
                nc.sync.dma_start(
                    x_all[DE : DE + H, seg : seg + BLK], hT_d[:, bsl]
                )

                # sh[p, f] = h[p // 8, f]: one DMA, 8x replication via a
                # stride-0 free dim on the 16-partition source.
                sh = shpool.tile([128, BLK], f16, tag="sh")
                nc.gpsimd.dma_start(
                    sh[:],
                    _inject_dim(x_all[DE : DE + H, seg : seg + BLK], [0, 8]),
                )

                pm = pmpool.tile([128, F], f32, tag="pm")

                def reduce_chunk(c, xs, t_ga, t_gb):
                    wsl = slice(c * 128, (c + 1) * 128)
                    nc.tensor.matmul(
                        pm[:], wcomb_s[:, wsl], xs,
                        start=(c == 0), stop=False,
                    )
                    nc.tensor.matmul(
                        pm[:], ga_s[:, wsl], t_ga, start=False, stop=False
                    )
                    nc.tensor.matmul(
                        pm[:], gb_s[:, wsl], t_gb,
                        start=False, stop=(c == BC - 1),
                    )

                for p in range(BC // 2):
                    c0, c1 = 2 * p, 2 * p + 1
                    xs0 = x_all[:, seg + c0 * F : seg + (c0 + 1) * F]
                    xs1 = x_all[:, seg + c1 * F : seg + (c1 + 1) * F]

                    pa0 = papool.tile([128, 2 * F], f32, tag="pa")
                    nc.tensor.matmul(
                        pa0[:, 0:F], wa_s[:], xs0, start=True, stop=True
                    )
                    nc.tensor.matmul(
                        pa0[:, F : 2 * F], wb_s[:], xs0, start=True, stop=True
                    )
                    pa1 = papool.tile([128, 2 * F], f32, tag="pa")
                    nc.tensor.matmul(
                        pa1[:, 0:F], wa_s[:], xs1, start=True, stop=True
                    )
                    nc.tensor.matmul(
                        pa1[:, F : 2 * F], wb_s[:], xs1, start=True, stop=True
                    )

                    # direct for 5 of 12 pairs (x ~= 0.58 offloaded): with the
                    # paired TT the offload path is DVE-cheaper, shifting the
                    # DVE/ACT balance point down from x = 2/3.
                    if (b * (BC // 2) + p) * 5 % 12 < 5:
                        # direct pair: per-chunk DVE TT from PSUM (1x mode);
                        # B operand reads the sh chunk twice (stride-0 dim)
                        t0 = tpool.tile([128, 2 * F], f16, tag="t")
                        shv = _inject_dim(sh[:, c0 * F : (c0 + 1) * F], [0, 2])
                        nc.vector.tensor_tensor(t0[:], pa0[:], shv, mul)
                        t1 = tpool.tile([128, 2 * F], f16, tag="t")
                        shv = _inject_dim(sh[:, c1 * F : (c1 + 1) * F], [0, 2])
                        nc.vector.tensor_tensor(t1[:], pa1[:], shv, mul)
                        reduce_chunk(c0, xs0, t0[:, 0:F], t0[:, F : 2 * F])
                        reduce_chunk(c1, xs1, t1[:, 0:F], t1[:, F : 2 * F])
                    else:
                        # offloaded pair: ACT evacuates both chunks' PSUM with
                        # interleaved destinations -> cp2 = [a0|a1|b0|b1], so
                        # ONE [128,2048] DVE TT in 2x mode covers the pair
                        # (B = sh[c0:c0+2chunks] read twice). Halves the DVE
                        # instruction count on the offload path; pa lifetime
                        # is unchanged (freed by the copy, not the TT).
                        cp2 = tpool.tile([128, 4 * F], f16, tag="cp2")
                        base = cp2[:]
                        bd = [list(d) for d in base.ap]
                        for k, pak in ((0, pa0), (1, pa1)):
                            dst = AP(
                                base.tensor,
                                base.offset + k * F,
                                [bd[0], [2 * F, 2], [1, F]],
                            )
                            nc.scalar.copy(dst, pak[:])
                        t2 = tpool.tile([128, 4 * F], f16, tag="t2")
                        shv = _inject_dim(
                            sh[:, c0 * F : (c0 + 2) * F], [0, 2]
                        )
                        nc.vector.tensor_tensor(t2[:], cp2[:], shv, mul)
                        reduce_chunk(c0, xs0, t2[:, 0:F], t2[:, 2 * F : 3 * F])
                        reduce_chunk(
                            c1, xs1, t2[:, F : 2 * F], t2[:, 3 * F : 4 * F]
                        )

                mo = mopool.tile([128, F], f16, tag="mo")
                nc.scalar.copy(mo[:], pm[:])
                nc.scalar.dma_start(mT_d[:, b * F : (b + 1) * F], mo[:])

    nc.compile()
    return nc


def host_prep_weights(W1, b1, W2, b2):
    """Dense weights -> device stationary tensors (fp16, j-major layout)."""
    W1 = np.asarray(W1, np.float32)
    b1 = np.asarray(b1, np.float32)
    W2 = np.asarray(W2, np.float32)
    b2 = np.asarray(b2, np.float32)

    p = np.arange(128)
    jj, il = p // 8, p % 8

    wa = np.zeros((KP, 128), np.float32)
    wb = np.zeros((KP, 128), np.float32)
    wa[:DE, :] = W1[:, il * H + jj]
    wb[:DE, :] = W1[:, (il + 8) * H + jj]

    b1r = b1.reshape(H, H).T  # b1r[j, i] = b1[i*H + j]
    wcomb = np.zeros((KP, BC * 128), np.float32)
    ga = np.zeros((128, BC * 128), np.float32)
    gb = np.zeros((128, BC * 128), np.float32)
    for c in range(BC):
        # columns q (0..127) of variant c live at wcomb[:, c*128 + q];
        # chunk c writes output partitions q = 16c + i
        q = H * c + np.arange(H)
        wcomb[0:DE, c * 128 + q] = W2
        wcomb[DE : DE + H, c * 128 + q] = b1r
        wcomb[DE + H, c * 128 + q] = b2
        ga[p, c * 128 + H * c + il] = 1.0
        gb[p, c * 128 + H * c + 8 + il] = 1.0

    pad = np.zeros((KP - DE - H, BLK), np.float16)
    pad[0, :] = 1.0  # ones row for b2 / W2 bias path

    return dict(
        wa=wa.astype(np.float16),
        wb=wb.astype(np.float16),
        wcomb=wcomb.astype(np.float16),
        ga=ga.astype(np.float16),
        gb=gb.astype(np.float16),
        pad=pad,
    )


def host_prep_inputs(h, e, Ec_pad):
    """Full [E,*] inputs -> per-core transposed fp16 arrays, padded."""
    E = e.shape[0]
    per = E // NCORES
    eT = np.zeros((NCORES, DE, Ec_pad), np.float16)
    hT = np.zeros((NCORES, H, Ec_pad), np.float16)
    e3 = np.asarray(e, np.float32).reshape(NCORES, per, DE)
    h3 = np.asarray(h, np.float32).reshape(NCORES, per, H)
    eT[:, :, :per] = e3.transpose(0, 2, 1).astype(np.float16)
    hT[:, :, :per] = h3.transpose(0, 2, 1).astype(np.float16)
    return eT, hT


def unpack_output(mT_all, E):
    """mT per core [128, Ec//8] fp16 -> full [E, H] fp32.

    mT[16c + i, b*F + f] = m(edge b*BLK + c*F + f, i)
    """
    per = E // NCORES
    out = np.empty((E, H), np.float32)
    for core in range(NCORES):
        mT = np.asarray(mT_all[core], np.float32)  # [128, nblk*F]
        nb = mT.shape[1] // F
        m = mT.reshape(BC, H, nb, F).transpose(2, 0, 3, 1).reshape(-1, H)
        out[core * per : (core + 1) * per] = m[:per]
    return out


_CACHE = {}


def _get_program(nblk):
    if nblk not in _CACHE:
        _CACHE[nblk] = build_program(nblk)
    return _CACHE[nblk]


def kernel(h, e, W1, b1, W2, b2):
    e = np.asarray(e)
    E = e.shape[0]
    assert E % NCORES == 0
    per = E // NCORES
    nblk = (per + BLK - 1) // BLK
    Ec_pad = nblk * BLK

    nc = _get_program(nblk)
    w = host_prep_weights(W1, b1, W2, b2)
    eT, hT = host_prep_inputs(h, e, Ec_pad)

    in_maps = [dict(eT=eT[c], hT=hT[c], **w) for c in range(NCORES)]
    res = run_bass_kernel_spmd(nc, in_maps, core_ids=list(range(NCORES)))
    return unpack_output([res.results[c]["mT"] for c in range(NCORES)], E)


# revision 23
# speedup vs baseline: 1.0486x; 1.0486x over previous
"""Trainium2 Bass kernel for nn_MessageLayer (GNN message passing).

Reference computation (per edge, E=1.6M, H=16, DE=32):
    A = (e @ W1 + b1).reshape(E, 16, 16)
    out[e,i] = sum_j A[e,i,j] * h[e,j]  +  (e @ W2 + b2)[e,i]

Pure data-parallel over E across 8 cores. Per core, edges are processed in
blocks of 4096 = 8 chunks x F=512 (one PSUM bank of fp32 per matmul output).

Partition layout ("j-major"): SBUF/PSUM partition p <-> (j, il) with
j = p//8, il = p%8.  Per chunk (512 edges):

  PE:  pa[:,0:512]   = wa^T @ x[0:65]   A^T half a (i=il),    PSUM bank pair
       pa[:,512:1024]= wb^T @ x[0:65]   A^T half b (i=il+8)
  t = pa * sh over [128,1024]; sh[p] = h[p//8] is read twice per chunk via a
       stride-0 middle AP dim. 1 of 3 chunks: direct DVE tensor_tensor from
       PSUM (1x); else ACT first evacuates pa to fp16 SBUF so the DVE
       multiply runs in 2x mode -- balances DVE vs ACT busy time.
  PE into pm [128,512] (24 matmuls per block accumulate, chunk c writes
       partition group 16c..16c+15 via shifted indicator weights):
       wcomb_c^T @ x[0:65]  (e@W2 + b1-term via h rows + b2 via ones row)
     + ga_c^T @ t[:,0:512]  (sum_j over half a)
     + gb_c^T @ t[:,512:]   (sum_j over half b)
  ACT: mo = copy(pm) fp16; one [128,512] copy + one DMA per 4096 edges.

All contraction dims are zero-padded to 65 (the pad rows of x are constant,
DMA'd once from a DRAM constant) and output dims to 128 so every matmul has
tile_size (128,128) -- no PE tiling-mode switches. Input DMAs are issued
from the SP queue, the sh broadcast (one DMA, stride-0 replication) from the
Pool queue, output copy + DMA from the ACT queue.

fp16 operands on the PE (fp32 matmul is 4x slower); PSUM accumulation fp32;
fp16 output (host upcasts). Measured rel-l2 error vs fp32 reference ~4e-4.
"""

import numpy as np

import concourse.mybir as mybir
import concourse.tile as tile
from concourse import bacc
from concourse.ap import AP
from concourse.bass_utils import run_bass_kernel_spmd

H = 16
DE = 32
NCORES = 8
F = 512          # edges per chunk (one PSUM bank of fp32)
BC = 8           # chunks per block
BLK = BC * F     # 4096 edges per block
KP = 65          # padded contraction dim: 32 e + 16 h + 1 ones + 16 zeros

f16 = mybir.dt.float16
f32 = mybir.dt.float32


def _inject_dim(ap, dim):
    """Return a copy of `ap` with `dim` ([stride, size]) inserted after the
    partition dim."""
    dims = [list(d) for d in ap.ap]
    return AP(ap.tensor, ap.offset, [dims[0], dim] + dims[1:])


def build_program(nblk: int):
    """SPMD Bass program for one core processing nblk*BLK edges."""
    Ec = nblk * BLK
    nc = bacc.Bacc("TRN2", target_bir_lowering=False, debug=False)

    eT_d = nc.dram_tensor("eT", [DE, Ec], f16, kind="ExternalInput")
    hT_d = nc.dram_tensor("hT", [H, Ec], f16, kind="ExternalInput")
    wa_d = nc.dram_tensor("wa", [KP, 128], f16, kind="ExternalInput")
    wb_d = nc.dram_tensor("wb", [KP, 128], f16, kind="ExternalInput")
    wcomb_d = nc.dram_tensor("wcomb", [KP, BC * 128], f16, kind="ExternalInput")
    ga_d = nc.dram_tensor("ga", [128, BC * 128], f16, kind="ExternalInput")
    gb_d = nc.dram_tensor("gb", [128, BC * 128], f16, kind="ExternalInput")
    pad_d = nc.dram_tensor("pad", [KP - DE - H, BLK], f16, kind="ExternalInput")
    mT_d = nc.dram_tensor("mT", [128, Ec // BC], f16, kind="ExternalOutput")

    mul = mybir.AluOpType.mult

    XBUFS = 3

    with tile.TileContext(nc) as tc:
        with (
            tc.tile_pool(name="const", bufs=1) as cpool,
            tc.tile_pool(name="sh", bufs=3) as shpool,
            tc.tile_pool(name="t", bufs=3) as tpool,
            tc.tile_pool(name="mo", bufs=2) as mopool,
            tc.tile_pool(name="pa", bufs=3, space="PSUM") as papool,
            tc.tile_pool(name="pm", bufs=2, space="PSUM") as pmpool,
        ):
            wa_s = cpool.tile([KP, 128], f16, tag="wa")
            wb_s = cpool.tile([KP, 128], f16, tag="wb")
            wcomb_s = cpool.tile([KP, BC * 128], f16, tag="wcomb")
            ga_s = cpool.tile([128, BC * 128], f16, tag="ga")
            gb_s = cpool.tile([128, BC * 128], f16, tag="gb")
            nc.scalar.dma_start(wa_s[:], wa_d[:])
            nc.scalar.dma_start(wb_s[:], wb_d[:])
            nc.scalar.dma_start(wcomb_s[:], wcomb_d[:])
            nc.scalar.dma_start(ga_s[:], ga_d[:])
            nc.scalar.dma_start(gb_s[:], gb_d[:])

            # x buffers: one persistent tile, XBUFS manually-rotated block
            # segments. Rows 48 (ones) and 49-64 (zeros) are constant pad --
            # initialized once, never rewritten, so every matmul can take
            # rhs = x[0:65] and run in the uniform (128,128) tile mode.
            x_all = cpool.tile([KP, XBUFS * BLK], f16, tag="x_all")
            for s in range(XBUFS):
                nc.scalar.dma_start(
                    x_all[DE + H : KP, s * BLK : (s + 1) * BLK], pad_d[:]
                )

            for b in range(nblk):
                bsl = slice(b * BLK, (b + 1) * BLK)
                seg = (b % XBUFS) * BLK
                nc.sync.dma_start(x_all[0:DE, seg : seg + BLK], eT_d[:, bsl])
                nc.sync.dma_start(
                    x_all[DE : DE + H, seg : seg + BLK], hT_d[:, bsl]
                )

                # sh[p, f] = h[p // 8, f]: one DMA, 8x replication via a
                # stride-0 free dim on the 16-partition source.
                sh = shpool.tile([128, BLK], f16, tag="sh")
                nc.gpsimd.dma_start(
                    sh[:],
                    _inject_dim(x_all[DE : DE + H, seg : seg + BLK], [0, 8]),
                )

                pm = pmpool.tile([128, F], f32, tag="pm")

                def reduce_chunk(c, xs, t_ga, t_gb):
                    wsl = slice(c * 128, (c + 1) * 128)
                    nc.tensor.matmul(
                        pm[:], wcomb_s[:, wsl], xs,
                        start=(c == 0), stop=False,
                    )
                    nc.tensor.matmul(
                        pm[:], ga_s[:, wsl], t_ga, start=False, stop=False
                    )
                    nc.tensor.matmul(
                        pm[:], gb_s[:, wsl], t_gb,
                        start=False, stop=(c == BC - 1),
                    )

                for p in range(BC // 2):
                    c0, c1 = 2 * p, 2 * p + 1
                    xs0 = x_all[:, seg + c0 * F : seg + (c0 + 1) * F]
                    xs1 = x_all[:, seg + c1 * F : seg + (c1 + 1) * F]

                    pa0 = papool.tile([128, 2 * F], f32, tag="pa")
                    nc.tensor.matmul(
                        pa0[:, 0:F], wa_s[:], xs0, start=True, stop=True
                    )
                    nc.tensor.matmul(
                        pa0[:, F : 2 * F], wb_s[:], xs0, start=True, stop=True
                    )
                    pa1 = papool.tile([128, 2 * F], f32, tag="pa")
                    nc.tensor.matmul(
                        pa1[:, 0:F], wa_s[:], xs1, start=True, stop=True
                    )
                    nc.tensor.matmul(
                        pa1[:, F : 2 * F], wb_s[:], xs1, start=True, stop=True
                    )

                    if (b * (BC // 2) + p) % 3 == 0:
                        # direct pair: per-chunk DVE TT from PSUM (1x mode);
                        # B operand reads the sh chunk twice (stride-0 dim)
                        t0 = tpool.tile([128, 2 * F], f16, tag="t")
                        shv = _inject_dim(sh[:, c0 * F : (c0 + 1) * F], [0, 2])
                        nc.vector.tensor_tensor(t0[:], pa0[:], shv, mul)
                        t1 = tpool.tile([128, 2 * F], f16, tag="t")
                        shv = _inject_dim(sh[:, c1 * F : (c1 + 1) * F], [0, 2])
                        nc.vector.tensor_tensor(t1[:], pa1[:], shv, mul)
                        reduce_chunk(c0, xs0, t0[:, 0:F], t0[:, F : 2 * F])
                        reduce_chunk(c1, xs1, t1[:, 0:F], t1[:, F : 2 * F])
                    else:
                        # offloaded pair: ACT evacuates both chunks' PSUM with
                        # interleaved destinations -> cp2 = [a0|a1|b0|b1], so
                        # ONE [128,2048] DVE TT in 2x mode covers the pair
                        # (B = sh[c0:c0+2chunks] read twice). Halves the DVE
                        # instruction count on the offload path; pa lifetime
                        # is unchanged (freed by the copy, not the TT).
                        cp2 = tpool.tile([128, 4 * F], f16, tag="cp2")
                        base = cp2[:]
                        bd = [list(d) for d in base.ap]
                        for k, pak in ((0, pa0), (1, pa1)):
                            dst = AP(
                                base.tensor,
                                base.offset + k * F,
                                [bd[0], [2 * F, 2], [1, F]],
                            )
                            nc.scalar.copy(dst, pak[:])
                        t2 = tpool.tile([128, 4 * F], f16, tag="t2")
                        shv = _inject_dim(
                            sh[:, c0 * F : (c0 + 2) * F], [0, 2]
                        )
                        nc.vector.tensor_tensor(t2[:], cp2[:], shv, mul)
                        reduce_chunk(c0, xs0, t2[:, 0:F], t2[:, 2 * F : 3 * F])
                        reduce_chunk(
                            c1, xs1, t2[:, F : 2 * F], t2[:, 3 * F : 4 * F]
                        )

                mo = mopool.tile([128, F], f16, tag="mo")
                nc.scalar.copy(mo[:], pm[:])
                nc.scalar.dma_start(mT_d[:, b * F : (b + 1) * F], mo[:])

    nc.compile()
    return nc


def host_prep_weights(W1, b1, W2, b2):
    """Dense weights -> device stationary tensors (fp16, j-major layout)."""
    W1 = np.asarray(W1, np.float32)
    b1 = np.asarray(b1, np.float32)
    W2 = np.asarray(W2, np.float32)
    b2 = np.asarray(b2, np.float32)

    p = np.arange(128)
    jj, il = p // 8, p % 8

    wa = np.zeros((KP, 128), np.float32)
    wb = np.zeros((KP, 128), np.float32)
    wa[:DE, :] = W1[:, il * H + jj]
    wb[:DE, :] = W1[:, (il + 8) * H + jj]

    b1r = b1.reshape(H, H).T  # b1r[j, i] = b1[i*H + j]
    wcomb = np.zeros((KP, BC * 128), np.float32)
    ga = np.zeros((128, BC * 128), np.float32)
    gb = np.zeros((128, BC * 128), np.float32)
    for c in range(BC):
        # columns q (0..127) of variant c live at wcomb[:, c*128 + q];
        # chunk c writes output partitions q = 16c + i
        q = H * c + np.arange(H)
        wcomb[0:DE, c * 128 + q] = W2
        wcomb[DE : DE + H, c * 128 + q] = b1r
        wcomb[DE + H, c * 128 + q] = b2
        ga[p, c * 128 + H * c + il] = 1.0
        gb[p, c * 128 + H * c + 8 + il] = 1.0

    pad = np.zeros((KP - DE - H, BLK), np.float16)
    pad[0, :] = 1.0  # ones row for b2 / W2 bias path

    return dict(
        wa=wa.astype(np.float16),
        wb=wb.astype(np.float16),
        wcomb=wcomb.astype(np.float16),
        ga=ga.astype(np.float16),
        gb=gb.astype(np.float16),
        pad=pad,
    )


def host_prep_inputs(h, e, Ec_pad):
    """Full [E,*] inputs -> per-core transposed fp16 arrays, padded."""
    E = e.shape[0]
    per = E // NCORES
    eT = np.zeros((NCORES, DE, Ec_pad), np.float16)
    hT = np.zeros((NCORES, H, Ec_pad), np.float16)
    e3 = np.asarray(e, np.float32).reshape(NCORES, per, DE)
    h3 = np.asarray(h, np.float32).reshape(NCORES, per, H)
    eT[:, :, :per] = e3.transpose(0, 2, 1).astype(np.float16)
    hT[:, :, :per] = h3.transpose(0, 2, 1).astype(np.float16)
    return eT, hT


def unpack_output(mT_all, E):
    """mT per core [128, Ec//8] fp16 -> full [E, H] fp32.

    mT[16c + i, b*F + f] = m(edge b*BLK + c*F + f, i)
    """
    per = E // NCORES
    out = np.empty((E, H), np.float32)
    for core in range(NCORES):
        mT = np.asarray(mT_all[core], np.float32)  # [128, nblk*F]
        nb = mT.shape[1] // F
        m = mT.reshape(BC, H, nb, F).transpose(2, 0, 3, 1).reshape(-1, H)
        out[core * per : (core + 1) * per] = m[:per]
    return out


_CACHE = {}


def _get_program(nblk):
    if nblk not in _CACHE:
        _CACHE[nblk] = build_program(nblk)
    return _CACHE[nblk]


def kernel(h, e, W1, b1, W2, b2):
    e = np.asarray(e)
    E = e.shape[0]
    assert E % NCORES == 0
    per = E // NCORES
    nblk = (per + BLK - 1) // BLK
    Ec_pad = nblk * BLK

    nc = _get_program(nblk)
    w = host_prep_weights(W1, b1, W2, b2)
    eT, hT = host_prep_inputs(h, e, Ec_pad)

    in_maps = [dict(eT=eT[c], hT=hT[c], **w) for c in range(NCORES)]
    res = run_bass_kernel_spmd(nc, in_maps, core_ids=list(range(NCORES)))
    return unpack_output([res.results[c]["mT"] for c in range(NCORES)], E)
